# revision 1
# baseline (speedup 1.0000x reference)
# Trainium2 Bass kernel for nn_CoefficientLayer (per-species MLP dispatch,
# ANI-style). Strategy: MoE routing done on host (sort atoms by species, pad
# each species group so all 8 cores get an identical tile schedule of 512/384/
# 256-atom tiles), device runs a dense 4-layer MLP per tile with the tile's
# species' weights selected statically at build time, emitted as a 4-deep
# software pipeline ([L1(t+3), L2(t+2), L3(t+1), L4(t)] per round) so the PE
# never stalls on the activation chain.
#
# Device math (per tile, feature-major, fp32r matmuls, fp32 psum), e = exp(1):
#   stored Hb_k = e*(elu(y_k)+1),  y_k = x_k/alpha,  using
#     e*(elu(y)+1) = max(e*(y+1), min(exp(y+1), e))
#   psum_k = e*(y_k+1) via lhsT chunks of plain W_k plus an augmented
#   bias row e*(beta_k+1) (own ones-row chunk for L1/L2; free zero-pad row of
#   the k1 chunk for L3).  Per hidden layer (both m-chunks consolidated):
#     ACT:  E = Exp(psum * (1/e))          [128, 2, n]
#     DVE:  Hb = (E min e) max psum        (scalar_tensor_tensor)
#   Zero-padded weight columns make pad rows of Hb exactly 1.0, which both
#   feeds L3's augmented row and is killed by zero rows in the next lhsT.
#   L4: psum4 = (alpha/e)*W4^T Hb3, coef = psum4 + alpha*beta4, then the
#   shifter folds into one ACT: out = s1*psum4 + (s0 + s1*alpha*beta4).
import numpy as np
from contextlib import ExitStack

import concourse.bass as bass
import concourse.tile as tile
from concourse import bacc, mybir
from concourse.bass_utils import run_bass_kernel_spmd

ALPHA = 0.1
E1 = float(np.exp(1.0))
P = 128
NCORES = 8
QUANTUM = 128
DIN = 384
DIMS = [384, 256, 192, 160]

F32 = mybir.dt.float32
F32R = mybir.dt.float32r
AF = mybir.ActivationFunctionType
ALU = mybir.AluOpType

# layer -> (n_weight_chunks_per_m (incl aug), n_m_chunks, true_K, true_M)
CHUNKS = {1: (4, 2, 384, 256), 2: (3, 2, 256, 192), 3: (2, 2, 192, 160),
          4: (2, 1, 160, 1)}
WCOLS_PER_S = (8 + 6 + 4) * P + 2
BCOLS_PER_S = 1  # shifter bias


def _wcol(s, layer, m, k):
    off = s * WCOLS_PER_S
    for l in (1, 2, 3):
        nk, nm = CHUNKS[l][0], CHUNKS[l][1]
        if l == layer:
            return off + (m * nk + k) * P
        off += nk * nm * P
    assert layer == 4 and m == 0
    return off + k


def _fold_host(inputs):
    """Pack weight image [128, 4*WCOLS_PER_S] and shifter consts."""
    al = ALPHA
    wimg = np.zeros((P, 4 * WCOLS_PER_S), dtype=np.float32)
    bimg = np.zeros((P, 4 * BCOLS_PER_S), dtype=np.float32)
    shifter_scale = []
    for s in range(4):
        W = [np.asarray(inputs[f"W{i}"][s], np.float32) for i in (1, 2, 3, 4)]
        b = [np.asarray(inputs[f"b{i}"][s], np.float32) for i in (1, 2, 3, 4)]
        Wt = [(E1 / al) * W[0], W[1], W[2], (al / E1) * W[3]]
        aug = [E1 * (b[0] / al + 1.0),
               E1 * (b[1] / al - W[1].sum(axis=0) + 1.0),
               E1 * (b[2] / al - W[2].sum(axis=0) + 1.0)]
        beta4 = b[3] - al * W[3].sum(axis=0)          # al*beta4 = b4 - al*colsum

        for layer in (1, 2, 3, 4):
            nk, nm, tk, tm = CHUNKS[layer]
            Wl = Wt[layer - 1]
            w = 1 if layer == 4 else P
            for m in range(nm):
                mlo, mhi = m * P, min((m + 1) * P, tm)
                for k in range(nk):
                    blk = np.zeros((P, w), np.float32)
                    is_aug = (layer in (1, 2)) and (k == nk - 1)
                    if is_aug:
                        blk[0, :mhi - mlo] = aug[layer - 1][mlo:mhi]
                    else:
                        rows = Wl[k * P:min((k + 1) * P, tk), mlo:mhi]
                        blk[:rows.shape[0], :rows.shape[1]] = rows
                        if layer == 3 and k == 1:
                            # aug row rides the zero-pad row 64 (Hb2m1 pad = 1)
                            blk[64, :mhi - mlo] = aug[2][mlo:mhi]
                    wimg[:, _wcol(s, layer, m, k):_wcol(s, layer, m, k) + w] = blk

        s1 = float(np.asarray(inputs["shift_b1"], np.float32)[s])
        s0 = float(np.asarray(inputs["shift_b0"], np.float32)[s])
        bimg[:, s] = s0 + s1 * float(beta4[0])
        shifter_scale.append(s1)
    return wimg, bimg, shifter_scale


def _host_prepare(inputs):
    species = np.asarray(inputs["species"]).ravel()
    aev = np.ascontiguousarray(np.asarray(inputs["aev"], np.float32).reshape(-1, DIN))
    order = np.argsort(species, kind="stable")
    counts = np.bincount(species, minlength=4)
    a = np.maximum(np.ceil(counts / (NCORES * QUANTUM)), 2).astype(int) * QUANTUM
    A_pc = int(a.sum())

    idx = np.full((NCORES, A_pc), -1, dtype=np.int64)
    off_sorted = 0
    off_core = 0
    for s in range(4):
        grp = order[off_sorted:off_sorted + counts[s]]
        for c in range(NCORES):
            lo = min(counts[s], c * a[s])
            hi = min(counts[s], (c + 1) * a[s])
            idx[c, off_core:off_core + (hi - lo)] = grp[lo:hi]
        off_sorted += counts[s]
        off_core += a[s]

    aev_t = np.zeros((NCORES, DIN, A_pc), dtype=np.float32)
    for c in range(NCORES):
        valid = idx[c] >= 0
        aev_t[c][:, valid] = aev[idx[c][valid]].T

    sched = []
    off = 0
    for s in range(4):
        rem = int(a[s])
        col = off
        while rem > 0:
            # keep every tile >= 256 atoms (fp32r full-rate needs N >= 256)
            if rem in (640, 384):
                n = rem - 256
            elif rem >= 512:
                n = 512
            else:
                n = rem
            assert n >= 256 or rem == n, (rem, n)
            sched.append((s, col, n))
            col += n
            rem -= n
        off += int(a[s])
    return aev_t, idx, sched, A_pc


def _build_program(sched, A_pc, shifter_scale):
    nc = bacc.Bacc("TRN2", target_bir_lowering=False, debug=False)
    aev_d = nc.dram_tensor("aev_t", [DIN, A_pc], F32R, kind="ExternalInput").ap()
    w_d = nc.dram_tensor("wimg", [P, 4 * WCOLS_PER_S], F32R, kind="ExternalInput").ap()
    b_d = nc.dram_tensor("bimg", [P, 4 * BCOLS_PER_S], F32, kind="ExternalInput").ap()
    out_d = nc.dram_tensor("out", [1, A_pc], F32, kind="ExternalOutput").ap()

    with tile.TileContext(nc) as tc, ExitStack() as ctx:
        wpool = ctx.enter_context(tc.tile_pool(name="w", bufs=1))
        xpool = ctx.enter_context(tc.tile_pool(name="x", bufs=4))
        hpool = ctx.enter_context(tc.tile_pool(name="h", bufs=3))
        epool = ctx.enter_context(tc.tile_pool(name="e", bufs=4))

        pspool = ctx.enter_context(tc.tile_pool(name="ps", bufs=1, space="PSUM"))
        ps4pool = ctx.enter_context(tc.tile_pool(name="ps4", bufs=2, space="PSUM"))

        # per-(species, layer) weight tiles; species 0's L1 chunks load
        # first (the prologue is DMA-bandwidth-bound), everything else is
        # deferred into the pipeline rounds
        lay_cols = {1: 8 * P, 2: 6 * P, 3: 4 * P, 4: 2}
        lay_off = {1: 0, 2: 8 * P, 3: 14 * P, 4: 18 * P}
        wtiles = {}

        def load_weights(sp, layers=(1, 2, 3, 4)):
            for ly in layers:
                wt = wpool.tile([P, lay_cols[ly]], F32R, tag=f"w{sp}L{ly}")
                c0 = sp * WCOLS_PER_S + lay_off[ly]
                nc.sync.dma_start(wt[:], w_d[:, c0:c0 + lay_cols[ly]])
                wtiles[(sp, ly)] = wt

        load_weights(0, layers=(1,))
        bsb = wpool.tile([P, 4 * BCOLS_PER_S], F32, tag="bimg")
        nc.sync.dma_start(bsb[:], b_d[:])
        ystage = wpool.tile([1, A_pc], F32, tag="ystage")
        ones_f = wpool.tile([P, 512], F32, tag="ones_f")
        nc.vector.memset(ones_f[:], 1.0)
        ones = wpool.tile([P, 512], F32R, tag="ones")
        nc.vector.tensor_copy(ones[:], ones_f[:])

        def wsl(s, layer, m, k, width=P):
            c0 = _wcol(s, layer, m, k) - s * WCOLS_PER_S - lay_off[layer]
            return wtiles[(s, layer)][:, c0:c0 + width]

        # 4-deep software pipeline: in one emission round the PE stream is
        # [L1(t+3), L2(t+2), L3(t+1), L4(t)], so each layer's exp->stt chain
        # elapses while the PE runs the other tiles' matmuls (no PE stalls).
        T = len(sched)
        hid = {}   # (tile, layer) -> hidden tile handle
        xloads = {}  # tile -> list of SBUF x-chunk APs

        def stage_load(t):
            s, col, n = sched[t]
            xt = xpool.tile([P, 3, 512], F32R, tag="x")
            src = aev_d.rearrange("(k p) a -> p k a", k=3)
            nc.sync.dma_start(xt[:, :, :n], src[:, :, col:col + n])
            xloads[t] = [xt[:, k, :n] for k in range(3)]

        def stage_hidden(t, layer):
            """Matmuls + exp + stt for `layer` (1..3) of tile t."""
            s, col, n = sched[t]
            if layer == 1:
                hs = xloads.pop(t)
            else:
                prev = hid.pop((t, layer - 1))
                hs = [prev[:, 0, :n], prev[:, 1, :n]]
            nk, nm, tk, tm = CHUNKS[layer]
            n_real = nk - 1 if layer in (1, 2) else nk
            ps = pspool.tile([P, 2, 512], F32, tag=f"ps{layer}")
            for m in range(nm):
                for k in range(n_real):
                    nc.tensor.matmul(ps[:, m, :n], wsl(s, layer, m, k), hs[k],
                                     start=(k == 0),
                                     stop=(k == n_real - 1 and layer == 3))
                if layer in (1, 2):  # augmented ones-row bias chunk
                    nc.tensor.matmul(ps[:, m, :n], wsl(s, layer, m, n_real),
                                     ones[:, :n], start=False, stop=True)
            et = epool.tile([P, 2, 512], F32, tag="e")
            nc.scalar.activation(et[:, :, :n], ps[:, :, :n], AF.Exp,
                                 bias=0.0, scale=1.0 / E1)
            ht = hpool.tile([P, 2, 512], F32R, tag=f"h{layer}")
            nc.vector.scalar_tensor_tensor(
                ht[:, :, :n], et[:, :, :n], E1, ps[:, :, :n],
                ALU.min, ALU.max)
            hid[(t, layer)] = ht

        species_last = {}
        species_range = {}
        for i, (sp, c0, nn_) in enumerate(sched):
            species_last[sp] = i
            lo, hi = species_range.get(sp, (c0, c0))
            species_range[sp] = (min(lo, c0), max(hi, c0 + nn_))

        def stage_out(t):
            """L4 matmuls + shifter + output flush for tile t."""
            s, col, n = sched[t]
            h3 = hid.pop((t, 3))
            ps4 = ps4pool.tile([1, 512], F32, tag="ps4")
            nc.tensor.matmul(ps4[:, :n], wsl(s, 4, 0, 0, width=1),
                             h3[:, 0, :n], start=True, stop=False)
            nc.tensor.matmul(ps4[:, :n], wsl(s, 4, 0, 1, width=1),
                             h3[:, 1, :n], start=False, stop=True)
            nc.scalar.activation(ystage[:, col:col + n], ps4[:, :n],
                                 AF.Identity,
                                 bias=bsb[0:1, s:s + 1], scale=shifter_scale[s])
            if species_last[s] == t:  # flush this species' outputs (overlaps)
                lo, hi = species_range[s]
                nc.sync.dma_start(out_d[:, lo:hi], ystage[:, lo:hi])

        # species s first needed at the L1 stage of its first tile; emit its
        # weight load ~3 rounds earlier
        first_tile = {}
        for i, (sp, _, _) in enumerate(sched):
            first_tile.setdefault(sp, i)
        wload_round = {max(-4, first_tile[sp] - 3 - 3): sp
                       for sp in sorted(first_tile) if sp != 0}

        for t in range(-5, T):
            if t == -3:
                load_weights(0, layers=(2, 3, 4))
            if t in wload_round:
                load_weights(wload_round[t])
            if 0 <= t + 5 < T:
                stage_load(t + 5)
            if 0 <= t + 3 < T:
                stage_hidden(t + 3, 1)
            if 0 <= t + 2 < T:
                stage_hidden(t + 2, 2)
            if 0 <= t + 1 < T:
                stage_hidden(t + 1, 3)
            if 0 <= t < T:
                stage_out(t)

    nc.compile()
    return nc


def kernel(**inputs):
    species = np.asarray(inputs["species"])
    out_dtype = np.asarray(inputs["aev"]).dtype
    aev_t, idx, sched, A_pc = _host_prepare(inputs)
    wimg, bimg, shifter_scale = _fold_host(inputs)
    nc = _build_program(sched, A_pc, shifter_scale)

    in_maps = [{"aev_t": np.ascontiguousarray(aev_t[c]), "wimg": wimg, "bimg": bimg}
               for c in range(NCORES)]
    res = run_bass_kernel_spmd(nc, in_maps, core_ids=list(range(NCORES)))

    out = np.zeros(species.size, dtype=np.float32)
    for c in range(NCORES):
        valid = idx[c] >= 0
        out[idx[c][valid]] = res.results[c]["out"][0][valid]
    return out.reshape(species.shape).astype(out_dtype, copy=False)



# revision 3
# speedup vs baseline: 1.0351x; 1.0351x over previous
# Trainium2 Bass kernel for nn_CoefficientLayer (per-species MLP dispatch,
# ANI-style). Strategy: MoE routing done on host (sort atoms by species, pad
# each species group so all 8 cores get an identical tile schedule of 512/384/
# 256-atom tiles), device runs a dense 4-layer MLP per tile with the tile's
# species' weights selected statically at build time, emitted as a 4-deep
# software pipeline ([L1(t+3), L2(t+2), L3(t+1), L4(t)] per round) so the PE
# never stalls on the activation chain.
#
# Device math (per tile, feature-major, fp32r matmuls, fp32 psum), e = exp(1):
#   stored Hb_k = e*(elu(y_k)+1),  y_k = x_k/alpha,  using
#     e*(elu(y)+1) = max(e*(y+1), min(exp(y+1), e))
#   psum_k = e*(y_k+1) via lhsT chunks of plain W_k plus an augmented
#   bias row e*(beta_k+1) (own ones-row chunk for L1/L2; free zero-pad row of
#   the k1 chunk for L3).  Per hidden layer (both m-chunks consolidated):
#     ACT:  E = Exp(psum * (1/e))          [128, 2, n]
#     DVE:  Hb = (E min e) max psum        (scalar_tensor_tensor)
#   Zero-padded weight columns make pad rows of Hb exactly 1.0, which both
#   feeds L3's augmented row and is killed by zero rows in the next lhsT.
#   L4: psum4 = (alpha/e)*W4^T Hb3, coef = psum4 + alpha*beta4, then the
#   shifter folds into one ACT: out = s1*psum4 + (s0 + s1*alpha*beta4).
import numpy as np
from contextlib import ExitStack

import concourse.bass as bass
import concourse.tile as tile
from concourse import bacc, mybir
from concourse.bass_utils import run_bass_kernel_spmd

ALPHA = 0.1
E1 = float(np.exp(1.0))
P = 128
NCORES = 8
QUANTUM = 128
DIN = 384
DIMS = [384, 256, 192, 160]

F32 = mybir.dt.float32
F32R = mybir.dt.float32r
AF = mybir.ActivationFunctionType
ALU = mybir.AluOpType

# layer -> (n_weight_chunks_per_m (incl aug), n_m_chunks, true_K, true_M)
CHUNKS = {1: (4, 2, 384, 256), 2: (3, 2, 256, 192), 3: (2, 2, 192, 160),
          4: (2, 1, 160, 1)}
WCOLS_PER_S = (8 + 6 + 4) * P + 2
BCOLS_PER_S = 1  # shifter bias


def _wcol(s, layer, m, k):
    off = s * WCOLS_PER_S
    for l in (1, 2, 3):
        nk, nm = CHUNKS[l][0], CHUNKS[l][1]
        if l == layer:
            return off + (m * nk + k) * P
        off += nk * nm * P
    assert layer == 4 and m == 0
    return off + k


def _fold_host(inputs):
    """Pack weight image [128, 4*WCOLS_PER_S] and shifter consts."""
    al = ALPHA
    wimg = np.zeros((P, 4 * WCOLS_PER_S), dtype=np.float32)
    bimg = np.zeros((P, 4 * BCOLS_PER_S), dtype=np.float32)
    shifter_scale = []
    for s in range(4):
        W = [np.asarray(inputs[f"W{i}"][s], np.float32) for i in (1, 2, 3, 4)]
        b = [np.asarray(inputs[f"b{i}"][s], np.float32) for i in (1, 2, 3, 4)]
        Wt = [(E1 / al) * W[0], W[1], W[2], (al / E1) * W[3]]
        aug = [E1 * (b[0] / al + 1.0),
               E1 * (b[1] / al - W[1].sum(axis=0) + 1.0),
               E1 * (b[2] / al - W[2].sum(axis=0) + 1.0)]
        beta4 = b[3] - al * W[3].sum(axis=0)          # al*beta4 = b4 - al*colsum

        for layer in (1, 2, 3, 4):
            nk, nm, tk, tm = CHUNKS[layer]
            Wl = Wt[layer - 1]
            w = 1 if layer == 4 else P
            for m in range(nm):
                mlo, mhi = m * P, min((m + 1) * P, tm)
                for k in range(nk):
                    blk = np.zeros((P, w), np.float32)
                    is_aug = (layer in (1, 2)) and (k == nk - 1)
                    if is_aug:
                        blk[0, :mhi - mlo] = aug[layer - 1][mlo:mhi]
                    else:
                        rows = Wl[k * P:min((k + 1) * P, tk), mlo:mhi]
                        blk[:rows.shape[0], :rows.shape[1]] = rows
                        if layer == 3 and k == 1:
                            # aug row rides the zero-pad row 64 (Hb2m1 pad = 1)
                            blk[64, :mhi - mlo] = aug[2][mlo:mhi]
                    wimg[:, _wcol(s, layer, m, k):_wcol(s, layer, m, k) + w] = blk

        s1 = float(np.asarray(inputs["shift_b1"], np.float32)[s])
        s0 = float(np.asarray(inputs["shift_b0"], np.float32)[s])
        bimg[:, s] = s0 + s1 * float(beta4[0])
        shifter_scale.append(s1)
    return wimg, bimg, shifter_scale


def _host_prepare(inputs):
    species = np.asarray(inputs["species"]).ravel()
    aev = np.ascontiguousarray(np.asarray(inputs["aev"], np.float32).reshape(-1, DIN))
    order = np.argsort(species, kind="stable")
    counts = np.bincount(species, minlength=4)
    a = np.maximum(np.ceil(counts / (NCORES * QUANTUM)), 2).astype(int) * QUANTUM
    A_pc = int(a.sum())

    idx = np.full((NCORES, A_pc), -1, dtype=np.int64)
    off_sorted = 0
    off_core = 0
    for s in range(4):
        grp = order[off_sorted:off_sorted + counts[s]]
        for c in range(NCORES):
            lo = min(counts[s], c * a[s])
            hi = min(counts[s], (c + 1) * a[s])
            idx[c, off_core:off_core + (hi - lo)] = grp[lo:hi]
        off_sorted += counts[s]
        off_core += a[s]

    aev_t = np.zeros((NCORES, DIN, A_pc), dtype=np.float32)
    for c in range(NCORES):
        valid = idx[c] >= 0
        aev_t[c][:, valid] = aev[idx[c][valid]].T

    sched = []
    off = 0
    for s in range(4):
        rem = int(a[s])
        col = off
        while rem > 0:
            # keep every tile >= 256 atoms (fp32r full-rate needs N >= 256)
            if rem in (640, 384):
                n = rem - 256
            elif rem >= 512:
                n = 512
            else:
                n = rem
            assert n >= 256 or rem == n, (rem, n)
            sched.append((s, col, n))
            col += n
            rem -= n
        off += int(a[s])
    return aev_t, idx, sched, A_pc


def _build_program(sched, A_pc, shifter_scale):
    nc = bacc.Bacc("TRN2", target_bir_lowering=False, debug=False)
    aev_d = nc.dram_tensor("aev_t", [DIN, A_pc], F32R, kind="ExternalInput").ap()
    w_d = nc.dram_tensor("wimg", [P, 4 * WCOLS_PER_S], F32R, kind="ExternalInput").ap()
    b_d = nc.dram_tensor("bimg", [P, 4 * BCOLS_PER_S], F32, kind="ExternalInput").ap()
    out_d = nc.dram_tensor("out", [1, A_pc], F32, kind="ExternalOutput").ap()

    with tile.TileContext(nc) as tc, ExitStack() as ctx:
        wpool = ctx.enter_context(tc.tile_pool(name="w", bufs=1))
        xpool = ctx.enter_context(tc.tile_pool(name="x", bufs=4))
        hpool = ctx.enter_context(tc.tile_pool(name="h", bufs=3))
        epool = ctx.enter_context(tc.tile_pool(name="e", bufs=4))

        pspool = ctx.enter_context(tc.tile_pool(name="ps", bufs=1, space="PSUM"))
        ps4pool = ctx.enter_context(tc.tile_pool(name="ps4", bufs=2, space="PSUM"))

        # per-(species, layer) weight tiles; species 0's L1 chunks load
        # first (the prologue is DMA-bandwidth-bound), everything else is
        # deferred into the pipeline rounds
        lay_cols = {1: 8 * P, 2: 6 * P, 3: 4 * P, 4: 2}
        lay_off = {1: 0, 2: 8 * P, 3: 14 * P, 4: 18 * P}
        wtiles = {}

        def load_weights(sp, layers=(1, 2, 3, 4)):
            for ly in layers:
                wt = wpool.tile([P, lay_cols[ly]], F32R, tag=f"w{sp}L{ly}")
                c0 = sp * WCOLS_PER_S + lay_off[ly]
                nc.sync.dma_start(wt[:], w_d[:, c0:c0 + lay_cols[ly]])
                wtiles[(sp, ly)] = wt

        load_weights(0, layers=(1,))
        bsb = wpool.tile([P, 4 * BCOLS_PER_S], F32, tag="bimg")
        nc.sync.dma_start(bsb[:], b_d[:])
        ystage = wpool.tile([1, A_pc], F32, tag="ystage")
        ones_f = wpool.tile([P, 512], F32, tag="ones_f")
        nc.vector.memset(ones_f[:], 1.0)
        ones = wpool.tile([P, 512], F32R, tag="ones")
        nc.vector.tensor_copy(ones[:], ones_f[:])

        def wsl(s, layer, m, k, width=P):
            c0 = _wcol(s, layer, m, k) - s * WCOLS_PER_S - lay_off[layer]
            return wtiles[(s, layer)][:, c0:c0 + width]

        # 4-deep software pipeline: in one emission round the PE stream is
        # [L1(t+3), L2(t+2), L3(t+1), L4(t)], so each layer's exp->stt chain
        # elapses while the PE runs the other tiles' matmuls (no PE stalls).
        T = len(sched)
        hid = {}   # (tile, layer) -> hidden tile handle
        xloads = {}  # tile -> list of SBUF x-chunk APs

        def stage_load(t):
            s, col, n = sched[t]
            xt = xpool.tile([P, 3, 512], F32R, tag="x")
            src = aev_d.rearrange("(k p) a -> p k a", k=3)
            nc.sync.dma_start(xt[:, :, :n], src[:, :, col:col + n])
            xloads[t] = [xt[:, k, :n] for k in range(3)]

        def stage_hidden(t, layer):
            """Matmuls + exp + stt for `layer` (1..3) of tile t."""
            s, col, n = sched[t]
            if layer == 1:
                hs = xloads.pop(t)
            else:
                prev = hid.pop((t, layer - 1))
                hs = [prev[:, 0, :n], prev[:, 1, :n]]
            nk, nm, tk, tm = CHUNKS[layer]
            n_real = nk - 1 if layer in (1, 2) else nk
            ps = pspool.tile([P, 2, 512], F32, tag=f"ps{layer}")
            for m in range(nm):
                for k in range(n_real):
                    nc.tensor.matmul(ps[:, m, :n], wsl(s, layer, m, k), hs[k],
                                     start=(k == 0),
                                     stop=(k == n_real - 1 and layer == 3))
                if layer in (1, 2):  # augmented ones-row bias chunk
                    nc.tensor.matmul(ps[:, m, :n], wsl(s, layer, m, n_real),
                                     ones[:, :n], start=False, stop=True)
            et = epool.tile([P, 2, 512], F32, tag="e")
            nc.scalar.activation(et[:, :, :n], ps[:, :, :n], AF.Exp,
                                 bias=0.0, scale=1.0 / E1)
            ht = hpool.tile([P, 2, 512], F32R, tag=f"h{layer}")
            nc.vector.scalar_tensor_tensor(
                ht[:, :, :n], et[:, :, :n], E1, ps[:, :, :n],
                ALU.min, ALU.max)
            hid[(t, layer)] = ht

        species_last = {}
        species_range = {}
        for i, (sp, c0, nn_) in enumerate(sched):
            species_last[sp] = i
            lo, hi = species_range.get(sp, (c0, c0))
            species_range[sp] = (min(lo, c0), max(hi, c0 + nn_))

        def stage_out(t):
            """L4 matmuls + shifter + output flush for tile t."""
            s, col, n = sched[t]
            h3 = hid.pop((t, 3))
            ps4 = ps4pool.tile([1, 512], F32, tag="ps4")
            nc.tensor.matmul(ps4[:, :n], wsl(s, 4, 0, 0, width=1),
                             h3[:, 0, :n], start=True, stop=False)
            nc.tensor.matmul(ps4[:, :n], wsl(s, 4, 0, 1, width=1),
                             h3[:, 1, :n], start=False, stop=True)
            nc.scalar.activation(ystage[:, col:col + n], ps4[:, :n],
                                 AF.Identity,
                                 bias=bsb[0:1, s:s + 1], scale=shifter_scale[s])
            if species_last[s] == t:  # flush this species' outputs (overlaps)
                lo, hi = species_range[s]
                nc.sync.dma_start(out_d[:, lo:hi], ystage[:, lo:hi])

        # species s first needed at the L1 stage of its first tile; emit its
        # weight load ~3 rounds earlier
        first_tile = {}
        for i, (sp, _, _) in enumerate(sched):
            first_tile.setdefault(sp, i)
        wload_round = {max(-4, first_tile[sp] - 3 - 3): sp
                       for sp in sorted(first_tile) if sp != 0}

        for t in range(-5, T):
            if t == -3:
                load_weights(0, layers=(2, 3, 4))
            if t in wload_round:
                load_weights(wload_round[t])
            if 0 <= t + 5 < T:
                stage_load(t + 5)
            if 0 <= t + 3 < T:
                stage_hidden(t + 3, 1)
            if 0 <= t + 2 < T:
                stage_hidden(t + 2, 2)
            if 0 <= t + 1 < T:
                stage_hidden(t + 1, 3)
            if 0 <= t < T:
                stage_out(t)

    nc.compile()
    return nc


def kernel(**inputs):
    species = np.asarray(inputs["species"])
    out_dtype = np.asarray(inputs["aev"]).dtype
    aev_t, idx, sched, A_pc = _host_prepare(inputs)
    wimg, bimg, shifter_scale = _fold_host(inputs)
    nc = _build_program(sched, A_pc, shifter_scale)

    in_maps = [{"aev_t": np.ascontiguousarray(aev_t[c]), "wimg": wimg, "bimg": bimg}
               for c in range(NCORES)]
    res = run_bass_kernel_spmd(nc, in_maps, core_ids=list(range(NCORES)))

    out = np.zeros(species.size, dtype=np.float32)
    for c in range(NCORES):
        valid = idx[c] >= 0
        out[idx[c][valid]] = res.results[c]["out"][0][valid]
    return out.reshape(species.shape).astype(out_dtype, copy=False)



# revision 4
# speedup vs baseline: 1.2105x; 1.1695x over previous
# Trainium2 Bass kernel for nn_CoefficientLayer (per-species MLP dispatch,
# ANI-style), v3.  Host routes atoms (sort by species, pad so all 8 cores get
# an identical tile schedule); device runs the dense 4-layer MLP per tile
# with the tile's species' weights selected statically at build time.
#
# Math (v1's proven 2-op elementwise chain; e = exp(1)):
#   psum p = e*(y+1) accumulated WITH bias via aug matmul passes
#   ACT:  E = Exp(p / e)             [psum -> sbuf]
#   DVE:  S = (E min e) max p        [scalar_tensor_tensor, psum in1]
# where stored S = e*(elu(y)+1); zero weight pad columns make S_pad = 1.0
# exactly, which feeds L3's aug ride-along row and L4's shift constant.
#
# v3 speedups over v1 (116 us): stride-2 software pipeline (L1(r), L2(r-2),
# L3(r-4), L4(r-6) per round) so psum WAR chains never stall the PE;
# fp8e4m3 DoubleRow matmuls for L1/L2 (2 k-chunks per pass) with bf16 aug
# passes, bf16 L3/L4; atom-major L4 (lhsT = S3 atom slice, rhs = W4 chunk,
# psum [128 atoms, 1] per group) so the out stage is a tiny ACT Identity
# (+shift const via per-partition bias AP) into [128, n_groups]; host
# un-transposes the [128, NG] output image.
import numpy as np
from contextlib import ExitStack

import ml_dtypes
import concourse.bass as bass
import concourse.tile as tile
from concourse import bacc, mybir
from concourse.bass_utils import run_bass_kernel_spmd

ALPHA = 0.1
E1 = float(np.exp(1.0))
P = 128
NCORES = 8
QUANTUM = 128
DIN = 384
DIMS = [384, 256, 192, 160]

F32 = mybir.dt.float32
BF16 = mybir.dt.bfloat16
FP8 = mybir.dt.float8e4
AF = mybir.ActivationFunctionType
ALU = mybir.AluOpType
PM = mybir.MatmulPerfMode

NP_FP8 = mybir.dt.np(FP8)
NP_BF16 = ml_dtypes.bfloat16

# fp8 image columns per species: L1 only (pair 256 + single 128) x 2m = 768.
# (Hidden activations stay bf16: the e*(y+1) encoding amplifies fp8 error
# via the +e offset and 1/alpha scaling, so only the raw aev input is fp8.)
W8_PER_S = 768
W8_L1 = 0
# bf16 image columns per species: aug L1 (2m x 128) = 256, aug L2 = 256,
# L2 (2m x 2k x 128) = 512, L3 (2m x 2k x 128) = 512, L4 (2 cols)
WB_PER_S = 256 + 256 + 512 + 512 + 2
WB_AUG1 = 0
WB_AUG2 = 256
WB_L2 = 512
WB_L3 = 1024
WB_L4 = 1536


def _fold_host(inputs):
    """Pack fp8 weight image (L1/L2 DoubleRow pairs + singles), bf16 image
    (aug rows, L3 with ride-along aug, L4), and shifter consts [128, 4]."""
    al = ALPHA
    w8 = np.zeros((P, 4 * W8_PER_S), dtype=NP_FP8)
    wb = np.zeros((P, 4 * WB_PER_S), dtype=NP_BF16)
    c4img = np.zeros((P, 4), dtype=np.float32)
    for s in range(4):
        W = [np.asarray(inputs[f"W{i}"][s], np.float64) for i in (1, 2, 3, 4)]
        b = [np.asarray(inputs[f"b{i}"][s], np.float64) for i in (1, 2, 3, 4)]
        s1 = float(np.asarray(inputs["shift_b1"], np.float32)[s])
        s0 = float(np.asarray(inputs["shift_b0"], np.float32)[s])
        W1t = (E1 / al) * W[0]          # [384, 256]
        W2, W3 = W[1], W[2]             # [256,192], [192,160]
        W4t = s1 * (al / E1) * W[3]     # [160, 1]
        aug1 = E1 * (b[0] / al + 1.0)                      # [256]
        aug2 = E1 * (b[1] / al - W2.sum(axis=0) + 1.0)     # [192]
        aug3 = E1 * (b[2] / al - W3.sum(axis=0) + 1.0)     # [160]
        c4img[:, s] = s0 + s1 * float(b[3][0] - al * W[3].sum(axis=0)[0])

        o8 = s * W8_PER_S
        ob = s * WB_PER_S
        for m in range(2):
            mlo, mhi = m * P, (m + 1) * P
            # L1: k-pair (rows 0..255) + single (rows 256..383), fp8
            blk = W1t[:, mlo:mhi]                       # [384, 128]
            pair = np.zeros((P, 2, P), np.float64)
            pair[:, 0] = blk[0:128]
            pair[:, 1] = blk[128:256]
            c0 = o8 + W8_L1 + m * 384
            w8[:, c0:c0 + 256] = pair.reshape(P, 256).astype(NP_FP8)
            w8[:, c0 + 256:c0 + 384] = blk[256:384].astype(NP_FP8)
            # L1 aug (bf16): row 0 = aug1 values
            wb[0, ob + WB_AUG1 + m * P:ob + WB_AUG1 + m * P + P] = \
                aug1[mlo:mhi].astype(NP_BF16)
            # L2 (bf16): k0 rows 0..127, k1 rows 128..255; aug row bf16
            mhi2 = min(mhi, 192)
            for k in range(2):
                blk2 = np.zeros((P, P), np.float64)
                blk2[:, :mhi2 - mlo] = W2[k * P:(k + 1) * P, mlo:mhi2]
                c0 = ob + WB_L2 + (m * 2 + k) * P
                wb[:, c0:c0 + P] = blk2.astype(NP_BF16)
            aug2p = np.zeros(P, np.float64)
            aug2p[:mhi2 - mlo] = aug2[mlo:mhi2]
            wb[0, ob + WB_AUG2 + m * P:ob + WB_AUG2 + m * P + P] = \
                aug2p.astype(NP_BF16)
            # L3 (bf16): k0 rows 0..127, k1 rows 128..191 (+aug ride row 64)
            mhi3 = min(mhi, 160)
            for k in range(2):
                blk3 = np.zeros((P, P), np.float64)
                rows = W3[k * P:min((k + 1) * P, 192)]
                blk3[:rows.shape[0], :mhi3 - mlo] = rows[:, mlo:mhi3]
                if k == 1:
                    # S2 pad rows are exactly 1.0; row 64 carries aug3
                    blk3[64, :mhi3 - mlo] = aug3[mlo:mhi3]
                c0 = ob + WB_L3 + (m * 2 + k) * P
                wb[:, c0:c0 + P] = blk3.astype(NP_BF16)
        # L4 (bf16): 2 single-col chunks; S3 pads are 1.0 but the k1 matmul
        # only reads partitions 0:32, so no ride-along is possible -> the
        # shift const c4 goes through the out-stage ACT bias instead.
        w4c = np.zeros((P, 2), np.float64)
        w4c[:, 0] = W4t[0:128, 0]
        w4c[0:32, 1] = W4t[128:160, 0]
        wb[:, ob + WB_L4:ob + WB_L4 + 2] = w4c.astype(NP_BF16)
    return w8, wb, c4img


def _host_prepare(inputs):
    species = np.asarray(inputs["species"]).ravel()
    aev = np.ascontiguousarray(
        np.asarray(inputs["aev"], np.float32).reshape(-1, DIN))
    order = np.argsort(species, kind="stable")
    counts = np.bincount(species, minlength=4)
    a = np.maximum(np.ceil(counts / (NCORES * QUANTUM)), 1).astype(int) * QUANTUM
    A_pc = int(a.sum())

    idx = np.full((NCORES, A_pc), -1, dtype=np.int64)
    off_sorted = 0
    off_core = 0
    for s in range(4):
        grp = order[off_sorted:off_sorted + counts[s]]
        for c in range(NCORES):
            lo = min(counts[s], c * a[s])
            hi = min(counts[s], (c + 1) * a[s])
            idx[c, off_core:off_core + (hi - lo)] = grp[lo:hi]
        off_sorted += counts[s]
        off_core += a[s]

    aev_t = np.zeros((NCORES, DIN, A_pc), dtype=NP_FP8)
    for c in range(NCORES):
        valid = idx[c] >= 0
        aev_t[c][:, valid] = aev[idx[c][valid]].astype(NP_FP8).T

    sched = []
    off = 0
    for s in range(4):
        rem = int(a[s])
        col = off
        while rem > 0:
            n = 512 if rem >= 512 else rem
            sched.append((s, col, n))
            col += n
            rem -= n
        off += int(a[s])
    return aev_t, idx, sched, A_pc


def _build_program(sched, A_pc):
    NG = A_pc // P  # number of 128-atom output groups
    nc = bacc.Bacc("TRN2", target_bir_lowering=False, debug=False)
    aev_d = nc.dram_tensor("aev_t", [DIN, A_pc], FP8, kind="ExternalInput").ap()
    w8_d = nc.dram_tensor("w8img", [P, 4 * W8_PER_S], FP8,
                          kind="ExternalInput").ap()
    wb_d = nc.dram_tensor("wbimg", [P, 4 * WB_PER_S], BF16,
                          kind="ExternalInput").ap()
    c4_d = nc.dram_tensor("c4img", [P, 4], F32, kind="ExternalInput").ap()
    out_d = nc.dram_tensor("out", [P, NG], F32, kind="ExternalOutput").ap()

    with tile.TileContext(nc) as tc, ExitStack() as ctx:
        wpool = ctx.enter_context(tc.tile_pool(name="w", bufs=1))
        xpool = ctx.enter_context(tc.tile_pool(name="x", bufs=4))
        epool = ctx.enter_context(tc.tile_pool(name="e", bufs=2))
        hpool = ctx.enter_context(tc.tile_pool(name="h", bufs=3))
        pspool = ctx.enter_context(tc.tile_pool(name="ps", bufs=1, space="PSUM"))
        ps4pool = ctx.enter_context(tc.tile_pool(name="ps4", bufs=2, space="PSUM"))

        w8tiles = {}
        wbtiles = {}

        def load_weights(sp):
            t8 = wpool.tile([P, W8_PER_S], FP8, tag=f"w8_{sp}")
            nc.sync.dma_start(t8[:], w8_d[:, sp * W8_PER_S:(sp + 1) * W8_PER_S])
            w8tiles[sp] = t8
            tb = wpool.tile([P, WB_PER_S], BF16, tag=f"wb_{sp}")
            nc.sync.dma_start(tb[:], wb_d[:, sp * WB_PER_S:(sp + 1) * WB_PER_S])
            wbtiles[sp] = tb

        load_weights(0)
        c4sb = wpool.tile([P, 4], F32, tag="c4img")
        nc.sync.dma_start(c4sb[:], c4_d[:])
        ystage = wpool.tile([P, NG], F32, tag="ystage")
        ones_f = wpool.tile([P, 512], F32, tag="ones_f")
        nc.vector.memset(ones_f[:], 1.0)
        ones = wpool.tile([P, 512], BF16, tag="ones")
        nc.vector.tensor_copy(ones[:], ones_f[:])

        T = len(sched)
        hid = {}     # (tile, layer) -> S tile handle
        psums = {}   # (tile, layer) -> psum handle
        es = {}      # (tile, layer) -> E tile handle
        xloads = {}  # tile -> x tile handle
        H_DT = {1: BF16, 2: BF16, 3: BF16}

        def stage_load(t):
            s, col, n = sched[t]
            xt = xpool.tile([P, 3, 512], FP8, tag="x")
            src = aev_d.rearrange("(k p) a -> p k a", k=3)
            nc.sync.dma_start(xt[:, :, :n], src[:, :, col:col + n])
            xloads[t] = xt

        def stage_mm(t, layer):
            s, col, n = sched[t]
            ps = pspool.tile([P, 2, 512], F32, tag=f"ps{layer}")
            t8 = w8tiles[s]
            tb = wbtiles[s]
            if layer == 1:
                xt = xloads.pop(t)
                for m in range(2):
                    c0 = W8_L1 + m * 384
                    lhsT = t8[:, c0:c0 + 256].rearrange("p (k m) -> p k m", k=2)
                    nc.tensor.matmul(ps[:, m, :n], lhsT, xt[:, 0:2, :n],
                                     start=True, stop=False,
                                     perf_mode=PM.DoubleRow)
                    nc.tensor.matmul(ps[:, m, :n], t8[:, c0 + 256:c0 + 384],
                                     xt[:, 2, :n], start=False, stop=False)
                    nc.tensor.matmul(ps[:, m, :n],
                                     tb[:, WB_AUG1 + m * P:WB_AUG1 + (m + 1) * P],
                                     ones[:, :n], start=False, stop=True)
            elif layer == 2:
                h1 = hid.pop((t, 1))
                for m in range(2):
                    for k in range(2):
                        c0 = WB_L2 + (m * 2 + k) * P
                        nc.tensor.matmul(ps[:, m, :n], tb[:, c0:c0 + P],
                                         h1[:, k, :n],
                                         start=(k == 0), stop=False)
                    nc.tensor.matmul(ps[:, m, :n],
                                     tb[:, WB_AUG2 + m * P:WB_AUG2 + (m + 1) * P],
                                     ones[:, :n], start=False, stop=True)
            else:
                h2 = hid.pop((t, 2))
                for m in range(2):
                    for k in range(2):
                        c0 = WB_L3 + (m * 2 + k) * P
                        nc.tensor.matmul(ps[:, m, :n], tb[:, c0:c0 + P],
                                         h2[:, k, :n],
                                         start=(k == 0), stop=(k == 1))
            psums[(t, layer)] = ps

        def stage_act(t, layer):
            """ACT: E = Exp(psum / e)."""
            s, col, n = sched[t]
            ps = psums[(t, layer)]
            et = epool.tile([P, 2, 512], BF16, tag=f"e{layer}")
            nc.scalar.activation(et[:, :, :n], ps[:, :, :n], AF.Exp,
                                 bias=0.0, scale=1.0 / E1)
            es[(t, layer)] = et

        def stage_stt(t, layer):
            """DVE: S = (E min e) max psum; frees the psum."""
            s, col, n = sched[t]
            ps = psums.pop((t, layer))
            et = es.pop((t, layer))
            ht = hpool.tile([P, 2, 512], H_DT[layer], tag=f"h{layer}")
            nc.vector.scalar_tensor_tensor(
                ht[:, :, :n], et[:, :, :n], E1, ps[:, :, :n],
                ALU.min, ALU.max)
            hid[(t, layer)] = ht

        species_last = {}
        species_range = {}
        for i, (sp, c0, nn_) in enumerate(sched):
            species_last[sp] = i
            lo, hi = species_range.get(sp, (c0, c0))
            species_range[sp] = (min(lo, c0), max(hi, c0 + nn_))

        def stage_out(t):
            """L4 atom-major: psum [128 atoms, 1] per 128-atom group, then
            ACT Identity + per-species shift const into ystage."""
            s, col, n = sched[t]
            tb = wbtiles[s]
            h3 = hid.pop((t, 3))
            ng = n // P
            g0 = col // P
            ps4 = ps4pool.tile([P, 4, 1], F32, tag="ps4")
            for g in range(ng):
                nc.tensor.matmul(ps4[:, g, :], h3[:, 0, g * P:(g + 1) * P],
                                 tb[:, WB_L4:WB_L4 + 1],
                                 start=True, stop=False)
                nc.tensor.matmul(ps4[:, g, :],
                                 h3[0:32, 1, g * P:(g + 1) * P],
                                 tb[0:32, WB_L4 + 1:WB_L4 + 2],
                                 start=False, stop=True)
            nc.scalar.activation(ystage[:, g0:g0 + ng], ps4[:, 0:ng, 0],
                                 AF.Identity, bias=c4sb[:, s:s + 1], scale=1.0)
            if species_last[s] == t:  # flush this species' outputs (overlaps)
                lo, hi = species_range[s]
                nc.sync.dma_start(out_d[:, lo // P:hi // P],
                                  ystage[:, lo // P:hi // P])

        first_tile = {}
        for i, (sp, _, _) in enumerate(sched):
            first_tile.setdefault(sp, i)
        wload_round = {max(1, first_tile[sp] - 4): sp
                       for sp in sorted(first_tile) if sp != 0}

        # stride-2 software pipeline: per round r the PE runs
        # [L1(r), L2(r-2), L3(r-4), L4(r-6)]; each (tile, layer)'s
        # exp -> stt chain has ~2 rounds of slack before its consumer.
        for r in range(0, T + 7):
            if r in wload_round:
                load_weights(wload_round[r])
            if r == 0:
                for tt in range(min(4, T)):
                    stage_load(tt)
            elif r + 3 < T:
                stage_load(r + 3)
            for layer, t in ((1, r), (2, r - 2), (3, r - 4)):
                if 0 <= t < T:
                    stage_mm(t, layer)
            if 0 <= r - 6 < T:
                stage_out(r - 6)
            for layer, t in ((1, r), (2, r - 2), (3, r - 4)):
                if 0 <= t < T:
                    stage_act(t, layer)
            for layer, t in ((1, r), (2, r - 2), (3, r - 4)):
                if 0 <= t < T:
                    stage_stt(t, layer)

    nc.compile()
    return nc


def kernel(**inputs):
    species = np.asarray(inputs["species"])
    out_dtype = np.asarray(inputs["aev"]).dtype
    aev_t, idx, sched, A_pc = _host_prepare(inputs)
    w8, wb, c4img = _fold_host(inputs)
    nc = _build_program(sched, A_pc)

    in_maps = [{"aev_t": np.ascontiguousarray(aev_t[c]), "w8img": w8,
                "wbimg": wb, "c4img": c4img} for c in range(NCORES)]
    res = run_bass_kernel_spmd(nc, in_maps, core_ids=list(range(NCORES)))

    out = np.zeros(species.size, dtype=np.float32)
    for c in range(NCORES):
        valid = idx[c] >= 0
        # device out is [128, NG] atom-major: atom (g*128 + p) at [p, g]
        flat = res.results[c]["out"].T.reshape(-1)
        out[idx[c][valid]] = flat[valid]
    return out.reshape(species.shape).astype(out_dtype, copy=False)


# revision 6
# speedup vs baseline: 1.2289x; 1.0152x over previous
# Trainium2 Bass kernel for nn_CoefficientLayer (per-species MLP dispatch,
# ANI-style), v3.  Host routes atoms (sort by species, pad so all 8 cores get
# an identical tile schedule); device runs the dense 4-layer MLP per tile
# with the tile's species' weights selected statically at build time.
#
# Math (v1's proven 2-op elementwise chain; e = exp(1)):
#   psum p = e*(y+1) accumulated WITH bias via aug matmul passes
#   ACT:  E = Exp(p / e)             [psum -> sbuf]
#   DVE:  S = (E min e) max p        [scalar_tensor_tensor, psum in1]
# where stored S = e*(elu(y)+1); zero weight pad columns make S_pad = 1.0
# exactly, which feeds L3's aug ride-along row and L4's shift constant.
#
# v4 speedups over v1 (116 us -> 96 us): stride-2 software pipeline (L1(r),
# L2(r-2), L3(r-4), L4(r-6) per round) so psum WAR/elementwise chains have
# ~2 rounds of slack and never stall the PE; fp8e4m3 DoubleRow matmuls for
# L1 and L2 (2 k-chunks per pass, fp8 aev + fp8 S1) with bf16 aug passes,
# bf16 L3/L4 (S2/S3 stay bf16: the e*(y+1) encoding amplifies fp8 rounding
# via the 1/alpha scale); zero-pad atom routing (A_pc == ceil(counts/8)
# summed, equal-split tiles <= 512); out stage is ACT Identity with the
# per-species shift const as a [1,1] bias AP.  Measured walls per core:
# PE 71 us / ACT 64 / DVE 59 at ~60-70% occupancy under the device's
# ~0.73 power-throttle limit.
import numpy as np
from contextlib import ExitStack

import ml_dtypes
import concourse.bass as bass
import concourse.tile as tile
from concourse import bacc, mybir
from concourse.bass_utils import run_bass_kernel_spmd

ALPHA = 0.1
E1 = float(np.exp(1.0))
P = 128
NCORES = 8
QUANTUM = 128
DIN = 384
DIMS = [384, 256, 192, 160]

F32 = mybir.dt.float32
BF16 = mybir.dt.bfloat16
FP8 = mybir.dt.float8e4
AF = mybir.ActivationFunctionType
ALU = mybir.AluOpType
PM = mybir.MatmulPerfMode

NP_FP8 = mybir.dt.np(FP8)
NP_BF16 = ml_dtypes.bfloat16

# fp8 image columns per species: L1 only (pair 256 + single 128) x 2m = 768.
# (Hidden activations stay bf16: the e*(y+1) encoding amplifies fp8 error
# via the +e offset and 1/alpha scaling, so only the raw aev input is fp8.)
W8_PER_S = 768 + 512
W8_L1 = 0
W8_L2 = 768
# bf16 image columns per species: aug L1 (2m x 128) = 256, aug L2 = 256,
# L3 (2m x 2k x 128) = 512, L4 (2 cols)
WB_PER_S = 256 + 256 + 512 + 2
WB_AUG1 = 0
WB_AUG2 = 256
WB_L3 = 512
WB_L4 = 1024


def _fold_host(inputs):
    """Pack fp8 weight image (L1/L2 DoubleRow pairs + singles), bf16 image
    (aug rows, L3 with ride-along aug, L4), and shifter consts [128, 4]."""
    al = ALPHA
    w8 = np.zeros((P, 4 * W8_PER_S), dtype=NP_FP8)
    wb = np.zeros((P, 4 * WB_PER_S), dtype=NP_BF16)
    c4img = np.zeros((P, 4), dtype=np.float32)
    for s in range(4):
        W = [np.asarray(inputs[f"W{i}"][s], np.float64) for i in (1, 2, 3, 4)]
        b = [np.asarray(inputs[f"b{i}"][s], np.float64) for i in (1, 2, 3, 4)]
        s1 = float(np.asarray(inputs["shift_b1"], np.float32)[s])
        s0 = float(np.asarray(inputs["shift_b0"], np.float32)[s])
        W1t = (E1 / al) * W[0]          # [384, 256]
        W2, W3 = W[1], W[2]             # [256,192], [192,160]
        W4t = s1 * (al / E1) * W[3]     # [160, 1]
        W2q = np.asarray(W2.astype(NP_FP8), np.float64)
        W3q = np.asarray(W3.astype(NP_BF16), np.float64)
        aug1 = E1 * (b[0] / al + 1.0)                      # [256]
        aug2 = E1 * (b[1] / al - W2q.sum(axis=0) + 1.0)    # [192]
        aug3 = E1 * (b[2] / al - W3q.sum(axis=0) + 1.0)    # [160]
        c4img[:, s] = s0 + s1 * float(b[3][0] - al * W[3].sum(axis=0)[0])

        o8 = s * W8_PER_S
        ob = s * WB_PER_S
        for m in range(2):
            mlo, mhi = m * P, (m + 1) * P
            # L1: k-pair (rows 0..255) + single (rows 256..383), fp8
            blk = W1t[:, mlo:mhi]                       # [384, 128]
            pair = np.zeros((P, 2, P), np.float64)
            pair[:, 0] = blk[0:128]
            pair[:, 1] = blk[128:256]
            c0 = o8 + W8_L1 + m * 384
            w8[:, c0:c0 + 256] = pair.reshape(P, 256).astype(NP_FP8)
            w8[:, c0 + 256:c0 + 384] = blk[256:384].astype(NP_FP8)
            # L1 aug (bf16): row 0 = aug1 values
            wb[0, ob + WB_AUG1 + m * P:ob + WB_AUG1 + m * P + P] = \
                aug1[mlo:mhi].astype(NP_BF16)
            # L2: k-pair (rows 0..255) as fp8 DoubleRow; aug row bf16
            mhi2 = min(mhi, 192)
            blk2 = np.zeros((256, P), np.float64)
            blk2[:, :mhi2 - mlo] = W2q[:, mlo:mhi2]
            pair2 = np.zeros((P, 2, P), np.float64)
            pair2[:, 0] = blk2[0:128]
            pair2[:, 1] = blk2[128:256]
            c0 = o8 + W8_L2 + m * 256
            w8[:, c0:c0 + 256] = pair2.reshape(P, 256).astype(NP_FP8)
            aug2p = np.zeros(P, np.float64)
            aug2p[:mhi2 - mlo] = aug2[mlo:mhi2]
            wb[0, ob + WB_AUG2 + m * P:ob + WB_AUG2 + m * P + P] = \
                aug2p.astype(NP_BF16)
            # L3 (bf16): k0 rows 0..127, k1 rows 128..191 (+aug ride row 64)
            mhi3 = min(mhi, 160)
            for k in range(2):
                blk3 = np.zeros((P, P), np.float64)
                rows = W3q[k * P:min((k + 1) * P, 192)]
                blk3[:rows.shape[0], :mhi3 - mlo] = rows[:, mlo:mhi3]
                if k == 1:
                    # S2 pad rows are exactly 1.0; row 64 carries aug3
                    blk3[64, :mhi3 - mlo] = aug3[mlo:mhi3]
                c0 = ob + WB_L3 + (m * 2 + k) * P
                wb[:, c0:c0 + P] = blk3.astype(NP_BF16)
        # L4 (bf16): 2 single-col chunks; S3 pads are 1.0 but the k1 matmul
        # only reads partitions 0:32, so no ride-along is possible -> the
        # shift const c4 goes through the out-stage ACT bias instead.
        w4c = np.zeros((P, 2), np.float64)
        w4c[:, 0] = W4t[0:128, 0]
        w4c[0:32, 1] = W4t[128:160, 0]
        wb[:, ob + WB_L4:ob + WB_L4 + 2] = w4c.astype(NP_BF16)
    return w8, wb, c4img


def _host_prepare(inputs):
    species = np.asarray(inputs["species"]).ravel()
    aev = np.ascontiguousarray(
        np.asarray(inputs["aev"], np.float32).reshape(-1, DIN))
    order = np.argsort(species, kind="stable")
    counts = np.bincount(species, minlength=4)
    a = np.maximum(np.ceil(counts / NCORES), 1).astype(int)
    A_pc = int(a.sum())

    idx = np.full((NCORES, A_pc), -1, dtype=np.int64)
    off_sorted = 0
    off_core = 0
    for s in range(4):
        grp = order[off_sorted:off_sorted + counts[s]]
        for c in range(NCORES):
            lo = min(counts[s], c * a[s])
            hi = min(counts[s], (c + 1) * a[s])
            idx[c, off_core:off_core + (hi - lo)] = grp[lo:hi]
        off_sorted += counts[s]
        off_core += a[s]

    aev_t = np.zeros((NCORES, DIN, A_pc), dtype=NP_FP8)
    for c in range(NCORES):
        valid = idx[c] >= 0
        aev_t[c][:, valid] = aev[idx[c][valid]].astype(NP_FP8).T

    sched = []
    off = 0
    for s in range(4):
        tot = int(a[s])
        nt = -(-tot // 512)
        col = off
        for i in range(nt):
            n = tot // nt + (1 if i < tot % nt else 0)
            sched.append((s, col, n))
            col += n
        off += tot
    return aev_t, idx, sched, A_pc


def _build_program(sched, A_pc):
    nc = bacc.Bacc("TRN2", target_bir_lowering=False, debug=False)
    aev_d = nc.dram_tensor("aev_t", [DIN, A_pc], FP8, kind="ExternalInput").ap()
    w8_d = nc.dram_tensor("w8img", [P, 4 * W8_PER_S], FP8,
                          kind="ExternalInput").ap()
    wb_d = nc.dram_tensor("wbimg", [P, 4 * WB_PER_S], BF16,
                          kind="ExternalInput").ap()
    c4_d = nc.dram_tensor("c4img", [P, 4], F32, kind="ExternalInput").ap()
    out_d = nc.dram_tensor("out", [1, A_pc], F32, kind="ExternalOutput").ap()

    with tile.TileContext(nc) as tc, ExitStack() as ctx:
        wpool = ctx.enter_context(tc.tile_pool(name="w", bufs=1))
        xpool = ctx.enter_context(tc.tile_pool(name="x", bufs=4))
        epool = ctx.enter_context(tc.tile_pool(name="e", bufs=2))
        hpool = ctx.enter_context(tc.tile_pool(name="h", bufs=3))
        pspool = ctx.enter_context(tc.tile_pool(name="ps", bufs=1, space="PSUM"))
        ps4pool = ctx.enter_context(tc.tile_pool(name="ps4", bufs=2, space="PSUM"))

        w8tiles = {}
        wbtiles = {}

        def load_weights(sp):
            t8 = wpool.tile([P, W8_PER_S], FP8, tag=f"w8_{sp}")
            nc.sync.dma_start(t8[:], w8_d[:, sp * W8_PER_S:(sp + 1) * W8_PER_S])
            w8tiles[sp] = t8
            tb = wpool.tile([P, WB_PER_S], BF16, tag=f"wb_{sp}")
            nc.sync.dma_start(tb[:], wb_d[:, sp * WB_PER_S:(sp + 1) * WB_PER_S])
            wbtiles[sp] = tb

        load_weights(0)
        c4sb = wpool.tile([P, 4], F32, tag="c4img")
        nc.sync.dma_start(c4sb[:], c4_d[:])
        ystage = wpool.tile([1, A_pc], F32, tag="ystage")
        ones_f = wpool.tile([P, 512], F32, tag="ones_f")
        nc.vector.memset(ones_f[:], 1.0)
        ones = wpool.tile([P, 512], BF16, tag="ones")
        nc.vector.tensor_copy(ones[:], ones_f[:])

        T = len(sched)
        hid = {}     # (tile, layer) -> S tile handle
        psums = {}   # (tile, layer) -> psum handle
        es = {}      # (tile, layer) -> E tile handle
        xloads = {}  # tile -> x tile handle
        H_DT = {1: FP8, 2: BF16, 3: BF16}

        def stage_load(t):
            s, col, n = sched[t]
            xt = xpool.tile([P, 3, 512], FP8, tag="x")
            src = aev_d.rearrange("(k p) a -> p k a", k=3)
            nc.sync.dma_start(xt[:, :, :n], src[:, :, col:col + n])
            xloads[t] = xt

        def stage_mm(t, layer):
            s, col, n = sched[t]
            ps = pspool.tile([P, 2, 512], F32, tag=f"ps{layer}")
            t8 = w8tiles[s]
            tb = wbtiles[s]
            if layer == 1:
                xt = xloads.pop(t)
                for m in range(2):
                    c0 = W8_L1 + m * 384
                    lhsT = t8[:, c0:c0 + 256].rearrange("p (k m) -> p k m", k=2)
                    nc.tensor.matmul(ps[:, m, :n], lhsT, xt[:, 0:2, :n],
                                     start=True, stop=False,
                                     perf_mode=PM.DoubleRow)
                    nc.tensor.matmul(ps[:, m, :n], t8[:, c0 + 256:c0 + 384],
                                     xt[:, 2, :n], start=False, stop=False)
                    nc.tensor.matmul(ps[:, m, :n],
                                     tb[:, WB_AUG1 + m * P:WB_AUG1 + (m + 1) * P],
                                     ones[:, :n], start=False, stop=True)
            elif layer == 2:
                h1 = hid.pop((t, 1))
                for m in range(2):
                    c0 = W8_L2 + m * 256
                    lhsT = t8[:, c0:c0 + 256].rearrange("p (k m) -> p k m", k=2)
                    nc.tensor.matmul(ps[:, m, :n], lhsT, h1[:, 0:2, :n],
                                     start=True, stop=False,
                                     perf_mode=PM.DoubleRow)
                    nc.tensor.matmul(ps[:, m, :n],
                                     tb[:, WB_AUG2 + m * P:WB_AUG2 + (m + 1) * P],
                                     ones[:, :n], start=False, stop=True)
            else:
                h2 = hid.pop((t, 2))
                for m in range(2):
                    for k in range(2):
                        c0 = WB_L3 + (m * 2 + k) * P
                        nc.tensor.matmul(ps[:, m, :n], tb[:, c0:c0 + P],
                                         h2[:, k, :n],
                                         start=(k == 0), stop=(k == 1))
            psums[(t, layer)] = ps

        def stage_act(t, layer):
            """ACT: E = Exp(psum / e)."""
            s, col, n = sched[t]
            ps = psums[(t, layer)]
            et = epool.tile([P, 2, 512], BF16, tag=f"e{layer}")
            nc.scalar.activation(et[:, :, :n], ps[:, :, :n], AF.Exp,
                                 bias=0.0, scale=1.0 / E1)
            es[(t, layer)] = et

        def stage_stt(t, layer):
            """DVE: S = (E min e) max psum; frees the psum."""
            s, col, n = sched[t]
            ps = psums.pop((t, layer))
            et = es.pop((t, layer))
            ht = hpool.tile([P, 2, 512], H_DT[layer], tag=f"h{layer}")
            nc.vector.scalar_tensor_tensor(
                ht[:, :, :n], et[:, :, :n], E1, ps[:, :, :n],
                ALU.min, ALU.max)
            hid[(t, layer)] = ht

        species_last = {}
        species_range = {}
        for i, (sp, c0, nn_) in enumerate(sched):
            species_last[sp] = i
            lo, hi = species_range.get(sp, (c0, c0))
            species_range[sp] = (min(lo, c0), max(hi, c0 + nn_))

        def stage_out(t):
            """L4 feature-major: psum [1, n], then ACT Identity + shift
            const (per-partition [1,1] bias AP) into ystage."""
            s, col, n = sched[t]
            tb = wbtiles[s]
            h3 = hid.pop((t, 3))
            ps4 = ps4pool.tile([1, 512], F32, tag="ps4")
            nc.tensor.matmul(ps4[:, :n], tb[:, WB_L4:WB_L4 + 1],
                             h3[:, 0, :n], start=True, stop=False)
            nc.tensor.matmul(ps4[:, :n], tb[0:32, WB_L4 + 1:WB_L4 + 2],
                             h3[0:32, 1, :n], start=False, stop=True)
            nc.scalar.activation(ystage[:, col:col + n], ps4[:, :n],
                                 AF.Identity, bias=c4sb[0:1, s:s + 1],
                                 scale=1.0)
            if species_last[s] == t:  # flush this species' outputs (overlaps)
                lo, hi = species_range[s]
                nc.sync.dma_start(out_d[:, lo:hi], ystage[:, lo:hi])

        first_tile = {}
        for i, (sp, _, _) in enumerate(sched):
            first_tile.setdefault(sp, i)
        wload_round = {max(1, first_tile[sp] - 4): sp
                       for sp in sorted(first_tile) if sp != 0}

        # stride-2 software pipeline: per round r the PE runs
        # [L1(r), L2(r-2), L3(r-4), L4(r-6)]; each (tile, layer)'s
        # exp -> stt chain has ~2 rounds of slack before its consumer.
        for r in range(0, T + 7):
            if r in wload_round:
                load_weights(wload_round[r])
            if r == 0:
                for tt in range(min(4, T)):
                    stage_load(tt)
            elif r + 3 < T:
                stage_load(r + 3)
            for layer, t in ((1, r), (2, r - 2), (3, r - 4)):
                if 0 <= t < T:
                    stage_mm(t, layer)
            if 0 <= r - 6 < T:
                stage_out(r - 6)
            for layer, t in ((1, r), (2, r - 2), (3, r - 4)):
                if 0 <= t < T:
                    stage_act(t, layer)
            for layer, t in ((1, r), (2, r - 2), (3, r - 4)):
                if 0 <= t < T:
                    stage_stt(t, layer)

    nc.compile()
    return nc


def kernel(**inputs):
    species = np.asarray(inputs["species"])
    out_dtype = np.asarray(inputs["aev"]).dtype
    aev_t, idx, sched, A_pc = _host_prepare(inputs)
    w8, wb, c4img = _fold_host(inputs)
    nc = _build_program(sched, A_pc)

    in_maps = [{"aev_t": np.ascontiguousarray(aev_t[c]), "w8img": w8,
                "wbimg": wb, "c4img": c4img} for c in range(NCORES)]
    res = run_bass_kernel_spmd(nc, in_maps, core_ids=list(range(NCORES)))

    out = np.zeros(species.size, dtype=np.float32)
    for c in range(NCORES):
        valid = idx[c] >= 0
        out[idx[c][valid]] = res.results[c]["out"][0][valid]
    return out.reshape(species.shape).astype(out_dtype, copy=False)
